# revision 7
# baseline (speedup 1.0000x reference)
"""BehaviorMoE Trainium2 kernel (8 NeuronCores, SPMD data-parallel over sorted tokens).

Contract: kernel(**inputs) takes FULL inputs as returned by setup_inputs() and
returns the FULL [8192, 1024] float32 output.

Strategy:
  - Host: sort tokens by behavior id. Tokens with b==0 need no expert compute
    (output = x + beta); they are used as masked filler so that every core gets
    exactly 1024 tokens that share a single behavior id.  Each core receives
    the stacked weight matrix [W_sh0; W_sh1; W_sh2; W_sp[t]]^T for its behavior.
  - Device (identical SPMD program, per-core data):
      Phase B (gates): per 128-token tile, gate logits (PE), masked softmax
        (DVE/ACT), PE transpose of gates, bias combine via gates^T @ b_all (PE)
        copied into an SBUF accumulator.
      Phase C (experts): e-outer loop streams the stacked weights once while
        the PE runs a dense fp32r matmul stream; a fused DVE
        scalar_tensor_tensor accumulates gate-weighted expert outputs into
        ping-pong SBUF accumulators (in-place DVE ops fault on this HW).
      Phase D (tail): LayerNorm stats (ACT Square batched to avoid act-table
        reloads), normalize + residual, DMA out.
  - Host: scatter per-core outputs back to original token order.
"""

import os
import sys

import numpy as np

for _p in ("/opt/trn_rl_repo", "/root/.axon_site/_ro/trn_rl_repo"):
    if os.path.isdir(_p) and _p not in sys.path:
        sys.path.append(_p)

from contextlib import ExitStack

from concourse import bacc, bass, masks, mybir, tile
from concourse.bass_utils import run_bass_kernel_spmd

F32 = mybir.dt.float32
F32R = mybir.dt.float32r
AX = mybir.AxisListType
ALU = mybir.AluOpType
ACTF = mybir.ActivationFunctionType

D = 1024            # model dim
N = 8192            # tokens
NB = 4              # behaviors
NESH = 3            # shared experts
NE = 4              # experts per behavior (3 shared + 1 specific)
EPS = 1e-5
NCORES = 8
M = N // NCORES     # tokens per core
KT = D // 128       # k tiles (contraction)
IT = M // 128       # token tiles per core
FH = 512            # feature half-tile (psum bank width in f32)


def _build_program(trivial_affine: bool) -> bass.Bass:
    nc = bacc.Bacc()

    xt_d = nc.declare_dram_parameter("xt", [KT, 128, M], F32R, isOutput=False)
    xtok_d = nc.declare_dram_parameter("xtok", [M, D], F32, isOutput=False)
    wt_d = nc.declare_dram_parameter("wt", [NE, 2, KT, 128, FH], F32R, isOutput=False)
    wg_d = nc.declare_dram_parameter("wg", [128, KT * NE], F32R, isOutput=False)
    ball_d = nc.declare_dram_parameter("ball", [NE, D], F32R, isOutput=False)
    mask_d = nc.declare_dram_parameter("mask", [128, IT], F32, isOutput=False)
    if not trivial_affine:
        gam_d = nc.declare_dram_parameter("gam", [128, D], F32, isOutput=False)
        bet_d = nc.declare_dram_parameter("bet", [128, D], F32, isOutput=False)
    out_d = nc.declare_dram_parameter("out", [M, D], F32, isOutput=True)

    with tile.TileContext(nc) as tc, ExitStack() as ctx:
        const = ctx.enter_context(tc.tile_pool(name="const", bufs=1))
        xtp = ctx.enter_context(tc.tile_pool(name="xt", bufs=KT))
        wpool = ctx.enter_context(tc.tile_pool(name="w", bufs=20))
        selp = ctx.enter_context(tc.tile_pool(name="sel", bufs=2 * IT))
        xtokp = ctx.enter_context(tc.tile_pool(name="xtok", bufs=3))
        outp = ctx.enter_context(tc.tile_pool(name="outp", bufs=2))
        scrp = ctx.enter_context(tc.tile_pool(name="scr", bufs=3))
        gatep = ctx.enter_context(tc.tile_pool(name="gate", bufs=IT))
        smallp = ctx.enter_context(tc.tile_pool(name="small", bufs=40))
        zpool = ctx.enter_context(tc.tile_pool(name="z", bufs=3, space="PSUM"))
        pspool = ctx.enter_context(tc.tile_pool(name="ps", bufs=2, space="PSUM"))

        # ---- constants / small inputs ----
        identity = const.tile([128, 128], F32, tag="ident")
        masks.make_identity(nc, identity[:])
        wg_sb = const.tile([128, KT * NE], F32R, tag="wg")
        nc.sync.dma_start(wg_sb[:], wg_d[:])
        ball_sb = const.tile([NE, D], F32R, tag="ball")
        nc.sync.dma_start(ball_sb[:], ball_d[:])
        mask_sb = const.tile([128, IT], F32, tag="mask")
        nc.sync.dma_start(mask_sb[:], mask_d[:])
        if not trivial_affine:
            gam_sb = const.tile([128, D], F32, tag="gam")
            nc.sync.dma_start(gam_sb[:], gam_d[:])
            bet_sb = const.tile([128, D], F32, tag="bet")
            nc.sync.dma_start(bet_sb[:], bet_d[:])

        # ---- resident xT k-tiles (full token width) ----
        xT = []
        for k in range(KT):
            t = xtp.tile([128, M], F32R, tag="xt")
            nc.sync.dma_start(t[:], xt_d[k])
            xT.append(t)

        # ---- stream all stacked-weight half-tiles (e, c, k) ----
        w_sb = {}
        for e in range(NE):
            for c in (0, 1):
                for k in range(KT):
                    t = wpool.tile([128, FH], F32R, tag="w")
                    nc.sync.dma_start(t[:], wt_d[e, c, k])
                    w_sb[(e, c, k)] = t

        # ---- accumulators (ping-pong; in-place DVE ops fault) ----
        selA = [selp.tile([128, D], F32, tag="sel", name=f"selA{i}") for i in range(IT)]
        selB = [selp.tile([128, D], F32, tag="sel", name=f"selB{i}") for i in range(IT)]

        # ---- Phase B: gates + bias init per token tile ----
        gates_t = []
        for i in range(IT):
            isl = slice(i * 128, (i + 1) * 128)
            glp = pspool.tile([128, NE], F32, tag="ps")
            for k in range(KT):
                nc.tensor.matmul(
                    glp[:], xT[k][:, isl], wg_sb[:, k * NE:(k + 1) * NE],
                    start=(k == 0), stop=(k == KT - 1),
                )
            negmax = smallp.tile([128, 1], F32, tag="s1")
            nc.vector.tensor_reduce(
                negmax[:], glp[:], axis=AX.X, op=ALU.max, negate=True
            )
            exps = smallp.tile([128, NE], F32, tag="s4")
            expsum = smallp.tile([128, 1], F32, tag="s1")
            nc.scalar.activation(
                exps[:], glp[:], ACTF.Exp,
                bias=negmax[:], scale=1.0, accum_out=expsum[:],
            )
            rinv = smallp.tile([128, 1], F32, tag="s1")
            nc.vector.reciprocal(rinv[:], expsum[:])
            rm = smallp.tile([128, 1], F32, tag="s1")
            nc.vector.tensor_mul(rm[:], rinv[:], mask_sb[:, i:i + 1])
            gates = gatep.tile([128, NE], F32, tag="g")
            nc.vector.tensor_scalar_mul(gates[:], exps[:], rm[:])
            gates_t.append(gates)

            gtp = pspool.tile([NE, 128], F32, tag="ps")
            nc.tensor.transpose(gtp[:], gates[:], identity[:])
            gT = smallp.tile([NE, 128], F32R, tag="gT")
            nc.vector.tensor_copy(gT[:], gtp[:])

            bps = zpool.tile([128, D], F32, tag="z")
            for c in (0, 1):
                nc.tensor.matmul(
                    bps[:, c * FH:(c + 1) * FH], gT[:],
                    ball_sb[:, c * FH:(c + 1) * FH],
                    start=True, stop=True,
                )
            nc.scalar.copy(selA[i][:], bps[:])

        # ---- Phase C: expert matmul stream + gated accumulate ----
        hsum = [None] * IT
        src, dst = selA, selB
        for e in range(NE):
            for i in range(IT):
                isl = slice(i * 128, (i + 1) * 128)
                zt = zpool.tile([128, D], F32, tag="z")
                for c in (0, 1):
                    for k in range(KT):
                        nc.tensor.matmul(
                            zt[:, c * FH:(c + 1) * FH],
                            xT[k][:, isl],
                            w_sb[(e, c, k)][:],
                            start=(k == 0), stop=(k == KT - 1),
                        )
                if e == NE - 1:
                    hs = smallp.tile([128, 1], F32, tag="s1")
                    nc.vector.scalar_tensor_tensor(
                        dst[i][:], zt[:], gates_t[i][:, e:e + 1], src[i][:],
                        op0=ALU.mult, op1=ALU.add, accum_out=hs[:],
                    )
                    hsum[i] = hs
                else:
                    nc.vector.scalar_tensor_tensor(
                        dst[i][:], zt[:], gates_t[i][:, e:e + 1], src[i][:],
                        op0=ALU.mult, op1=ALU.add,
                    )
            src, dst = dst, src
        selF = src  # final accumulators after NE swaps

        # ---- Phase D: LayerNorm + residual, batched by ACT function ----
        sq = []
        s2 = []
        for i in range(IT):  # all ACT Square back to back (one table load)
            scr = scrp.tile([128, D], F32, tag="scr")
            sqi = smallp.tile([128, 1], F32, tag="s1")
            nc.scalar.activation(scr[:], selF[i][:], ACTF.Square, accum_out=sqi[:])
            sq.append(sqi)
            s2i = smallp.tile([128, 1], F32, tag="s1")
            nc.scalar.activation(s2i[:], hsum[i][:], ACTF.Square)
            s2.append(s2i)
        av = []
        for i in range(IT):
            varn = smallp.tile([128, 1], F32, tag="s1")
            nc.vector.scalar_tensor_tensor(
                varn[:], s2[i][:], -1.0 / D, sq[i][:], op0=ALU.mult, op1=ALU.add
            )
            avi = smallp.tile([128, 1], F32, tag="s1")
            nc.vector.tensor_scalar(
                avi[:], varn[:], 1.0 / D, EPS, op0=ALU.mult, op1=ALU.add
            )
            av.append(avi)
        sd = []
        for i in range(IT):  # all ACT Sqrt back to back
            sdi = smallp.tile([128, 1], F32, tag="s1")
            nc.scalar.sqrt(sdi[:], av[i][:])
            sd.append(sdi)
        rstd = []
        mb = []
        for i in range(IT):
            ri = smallp.tile([128, 1], F32, tag="s1")
            nc.vector.reciprocal(ri[:], sd[i][:])
            rstd.append(ri)
            mbt = smallp.tile([128, 1], F32, tag="s1")
            nc.vector.tensor_mul(mbt[:], hsum[i][:], ri[:])
            mbi = smallp.tile([128, 1], F32, tag="s1")
            nc.vector.tensor_scalar_mul(mbi[:], mbt[:], -1.0 / D)
            mb.append(mbi)
        lnt = []
        for i in range(IT):  # all ACT Identity back to back
            ln = scrp.tile([128, D], F32, tag="scr")
            nc.scalar.activation(
                ln[:], selF[i][:], ACTF.Identity, bias=mb[i][:], scale=rstd[i][:]
            )
            lnt.append(ln)
        for i in range(IT):
            xi = xtokp.tile([128, D], F32, tag="xtok")
            nc.sync.dma_start(xi[:], xtok_d[i * 128:(i + 1) * 128, :])
            outt = outp.tile([128, D], F32, tag="out")
            if trivial_affine:
                nc.vector.tensor_add(outt[:], lnt[i][:], xi[:])
            else:
                lng = scrp.tile([128, D], F32, tag="scr")
                nc.vector.tensor_mul(lng[:], lnt[i][:], gam_sb[:])
                lnb = scrp.tile([128, D], F32, tag="scr")
                nc.vector.tensor_add(lnb[:], lng[:], bet_sb[:])
                nc.vector.tensor_add(outt[:], lnb[:], xi[:])
            nc.sync.dma_start(out_d[i * 128:(i + 1) * 128, :], outt[:])

    nc.finalize()
    return nc


_PROGRAM_CACHE: dict = {}


def _get_program(trivial_affine: bool) -> bass.Bass:
    key = trivial_affine
    if key not in _PROGRAM_CACHE:
        _PROGRAM_CACHE[key] = _build_program(trivial_affine)
    return _PROGRAM_CACHE[key]


def _pack_tokens(b: np.ndarray):
    """Partition 8192 tokens into 8 chunks of 1024, each chunk holding tokens
    of a single behavior (1..4) plus masked b==0 filler."""
    idx0 = np.flatnonzero(b == 0)
    chunks = []
    for t in range(1, NB + 1):
        idxs = np.flatnonzero(b == t)
        for s in range(0, max(len(idxs), 1), M):
            part = idxs[s:s + M]
            if len(part) or not chunks:
                chunks.append((part, t))
    chunks = [(p, t) for (p, t) in chunks if len(p) > 0]
    if len(chunks) > NCORES:
        raise RuntimeError(
            f"token packing needs {len(chunks)} single-behavior chunks > {NCORES}"
        )
    while len(chunks) < NCORES:
        chunks.append((np.empty((0,), np.int64), 1))
    p0 = 0
    cores = []
    for part, t in chunks:
        need = M - len(part)
        fill = idx0[p0:p0 + need]
        p0 += need
        if len(fill) != need:
            raise RuntimeError("not enough b==0 filler tokens for packing")
        idx = np.concatenate([part.astype(np.int64), fill.astype(np.int64)])
        msk = np.zeros((M,), np.float32)
        msk[:len(part)] = 1.0
        cores.append((idx, msk, t))
    assert p0 == len(idx0)
    return cores


def _behavior_tensors(W_sh, b_sh, W_sp, b_sp, w_gates):
    per_t = {}
    W_sh_flat = W_sh.reshape(NESH * D, D)
    for t in range(1, NB + 1):
        Wall = np.concatenate([W_sh_flat, W_sp[t - 1:t].reshape(D, D)], axis=0)
        wT = np.ascontiguousarray(Wall.T)                      # [D, NE*D]
        wt_h = np.ascontiguousarray(
            wT.reshape(KT, 128, NE, 2, FH).transpose(2, 3, 0, 1, 4)
        )                                                      # [e, c, k, 128, FH]
        wg_h = np.ascontiguousarray(
            w_gates[t - 1].reshape(KT, 128, NE).transpose(1, 0, 2).reshape(128, KT * NE)
        )
        ball_h = np.ascontiguousarray(
            np.stack([b_sh[0], b_sh[1], b_sh[2], b_sp[t - 1]], axis=0)
        )                                                      # [4, D]
        per_t[t] = (wt_h, wg_h, ball_h)
    return per_t


def _prepare(x, b_seq, W_sh, b_sh, W_sp, b_sp, w_gates, gamma, beta):
    x = np.ascontiguousarray(np.asarray(x, dtype=np.float32))
    b = np.asarray(b_seq).astype(np.int64).ravel()
    W_sh = np.asarray(W_sh, dtype=np.float32)
    b_sh = np.asarray(b_sh, dtype=np.float32)
    W_sp = np.asarray(W_sp, dtype=np.float32)
    b_sp = np.asarray(b_sp, dtype=np.float32)
    w_gates = np.asarray(w_gates, dtype=np.float32)
    gamma = np.asarray(gamma, dtype=np.float32)
    beta = np.asarray(beta, dtype=np.float32)
    assert x.shape == (N, D) and b.shape == (N,)

    trivial = bool(np.all(gamma == 1.0) and np.all(beta == 0.0))
    cores = _pack_tokens(b)
    per_t = _behavior_tensors(W_sh, b_sh, W_sp, b_sp, w_gates)

    in_maps = []
    for idx, msk, t in cores:
        wt_h, wg_h, ball_h = per_t[t]
        xc = np.ascontiguousarray(x[idx])                      # [M, D]
        xt_h = np.ascontiguousarray(xc.T).reshape(KT, 128, M)  # [k, 128, M]
        m = {
            "xt": xt_h,
            "xtok": xc,
            "wt": wt_h,
            "wg": wg_h,
            "ball": ball_h,
            "mask": np.ascontiguousarray(msk.reshape(IT, 128).T),
        }
        if not trivial:
            m["gam"] = np.ascontiguousarray(np.broadcast_to(gamma, (128, D)))
            m["bet"] = np.ascontiguousarray(np.broadcast_to(beta, (128, D)))
        in_maps.append(m)
    return trivial, cores, in_maps


def kernel_with_results(trace: bool = False, **inputs):
    trivial, cores, in_maps = _prepare(**inputs)
    nc = _get_program(trivial)
    res = run_bass_kernel_spmd(
        nc, in_maps, list(range(NCORES)), trace=trace
    )
    out = np.empty((N, D), np.float32)
    for c, (idx, _msk, _t) in enumerate(cores):
        out[idx] = res.results[c]["out"]
    return out, res


def kernel(**inputs) -> np.ndarray:
    out, _ = kernel_with_results(trace=False, **inputs)
    return out
